# revision 15
# baseline (speedup 1.0000x reference)
"""Multi-head causal self-attention (B=2, S=4096, D=512, H=8) on 8 trn2 cores.

Sharding: batch*heads = 16 (b,h) pairs -> 2 heads per core (head-parallel,
qkv weight columns sharded per head group). Zero cross-core communication.

Per-core kernel (heads h0=2g, h1=2g+1 stacked on partition halves):
  - inputs: xt = X[b].T  (512, 4096),  w = [Wq|Wk|Wv] head cols (512, 384)
  - QT/KT: (128, 4096) with partitions 0-63 = head0 dims, 64-127 = head1
  - V: natural layout per 128-row j-tile, with an appended ones column so
    the AV matmul also produces the softmax denominator (row 64 of out).
  - scores computed transposed (keys on partitions) so softmax sum comes
    from the ones column; exp on ScalarE with scale=1/8 folded in; causal
    mask via 4 static mask tiles (DVE multiply) + range restriction.
  - AV accumulated in PSUM over j-tiles; result O.T (65, 512) transposed
    back via PE transpose in 128-col blocks; normalized with per-partition
    reciprocal of the denominator column; DMA'd out as full 512B rows.
All matmuls use float32r (full PE rate at free dim >= 256).
"""

import os
import sys

import numpy as np

for _p in ("/opt/trn_rl_repo", "/root/.axon_site/_ro/trn_rl_repo"):
    if os.path.isdir(_p) and _p not in sys.path:
        sys.path.append(_p)

import concourse.bass as bass
import concourse.tile as tile
from concourse import mybir
from concourse.masks import make_identity

F32 = mybir.dt.float32
F32R = mybir.dt.float32r
BF16 = mybir.dt.bfloat16

B, S, D, H = 2, 4096, 512, 8
HD = 64          # head dim
NHC = 2          # heads per core
P = 128          # partitions
KC = D // P      # 4 contraction chunks for the projection
IT = 512         # query-range width
NI = S // IT     # 8 query ranges
JT = 128         # key-tile width
NJ = S // JT     # 32 key tiles
SCALE = 1.0 / np.sqrt(HD)  # 0.125


def build_nc():
    nc = bass.Bass()
    xt = nc.declare_dram_parameter("xt", [D, S], BF16, isOutput=False)
    w = nc.declare_dram_parameter("w", [D, 3 * P], BF16, isOutput=False)
    out = nc.declare_dram_parameter("out", [S, NHC * HD], F32, isOutput=True)

    with tile.TileContext(nc) as tc:
        with (
            tc.tile_pool(name="singles", bufs=1) as singles,
            tc.tile_pool(name="epool", bufs=6) as epool,
            tc.tile_pool(name="otpool", bufs=4) as otpool,
            tc.tile_pool(name="outtp", bufs=3) as outtp,
            tc.tile_pool(name="rcpool", bufs=4) as rcpool,
            tc.tile_pool(name="ps_sc", bufs=2, space="PSUM") as ps_sc,
            tc.tile_pool(name="ps_av", bufs=2, space="PSUM") as ps_av,
            tc.tile_pool(name="ps_tr", bufs=2, space="PSUM") as ps_tr,
        ):
            # ---- resident tensors -------------------------------------
            xt_sb = singles.tile([P, KC, S], BF16, name="xt_sb")
            w_sb = singles.tile([P, KC, 3 * P], BF16, name="w_sb")
            qt = singles.tile([P, S], BF16, name="qt")
            kt = singles.tile([P, S], BF16, name="kt")
            # V per j-tile: [jt, 0:64] head0, [jt, 64] ones, [jt, 65:129]
            # head1, [jt, 129] ones
            v_sb = singles.tile([P, NJ, 130], BF16, name="v_sb")
            masks_f = singles.tile([P, 4, IT], F32, name="masks_f")
            masks = singles.tile([P, 4, IT], BF16, name="masks")
            ident = singles.tile([P, P], F32, name="ident")
            zbias = singles.tile([P, 1], F32, name="zbias")

            # ---- loads + constants ------------------------------------
            nc.sync.dma_start(
                out=w_sb, in_=w[:, :].rearrange("(c p) n -> p c n", p=P)
            )
            xt_r = xt[:, :].rearrange("(c p) s -> p c s", p=P)
            for c in range(KC):
                nc.sync.dma_start(out=xt_sb[:, c, :], in_=xt_r[:, c, :])

            make_identity(nc, ident)
            nc.vector.memset(zbias, 0.0)
            # ones columns for the denominator rows
            nc.vector.memset(v_sb[:, :, 64:65], 1.0)
            nc.vector.memset(v_sb[:, :, 129:130], 1.0)
            # mask k: keep (=1) iff x - p - 128k >= 0, else 0
            for k in range(4):
                nc.gpsimd.memset(masks_f[:, k, :], 1.0)
                nc.gpsimd.affine_select(
                    out=masks_f[:, k, :],
                    in_=masks_f[:, k, :],
                    compare_op=mybir.AluOpType.is_ge,
                    fill=0.0,
                    base=-JT * k,
                    pattern=[[1, IT]],
                    channel_multiplier=-1,
                )
            nc.vector.tensor_copy(masks, masks_f)

            # ---- projections ------------------------------------------
            # V first (small-N matmuls, poor PE duty) so the dense QT/KT
            # burst runs last and flows straight into attention warm.
            for j in range(NJ):
                ps_v = ps_av.tile([P, IT], F32, tag="av", name="ps_v")
                for c in range(KC):
                    nc.tensor.matmul(
                        ps_v[:, 0:P],
                        lhsT=xt_sb[:, c, j * JT : (j + 1) * JT],
                        rhs=w_sb[:, c, 2 * P : 3 * P],
                        start=(c == 0),
                        stop=(c == KC - 1),
                    )
                nc.vector.tensor_copy(v_sb[:, j, 0:64], ps_v[:, 0:64])
                nc.vector.tensor_copy(v_sb[:, j, 65:129], ps_v[:, 64:128])
            # QT/KT: out = w_chunk.T @ xt_chunk  -> [128 head-dims, 512]
            # Emitted just-in-time inside the attention loop (range t needs
            # qt slice t and kt slices <= t) so dense projection matmuls mix
            # into the attention stream instead of a serial phase.
            def project_qk(r):
                sl = slice(r * IT, (r + 1) * IT)
                ps_q = ps_sc.tile([P, 2 * IT], F32, tag="sc", name="ps_q")
                for c in range(KC):
                    nc.tensor.matmul(
                        ps_q[:, 0:IT],
                        lhsT=w_sb[:, c, 0:P],
                        rhs=xt_sb[:, c, sl],
                        start=(c == 0),
                        stop=(c == KC - 1),
                    )
                for c in range(KC):
                    nc.tensor.matmul(
                        ps_q[:, IT : 2 * IT],
                        lhsT=w_sb[:, c, P : 2 * P],
                        rhs=xt_sb[:, c, sl],
                        start=(c == 0),
                        stop=(c == KC - 1),
                    )
                nc.vector.tensor_copy(qt[:, sl], ps_q[:, 0:IT])
                nc.vector.tensor_copy(kt[:, sl], ps_q[:, IT : 2 * IT])

            # ---- attention --------------------------------------------
            exp_f = mybir.ActivationFunctionType.Exp

            def tail_block(ot_tiles, ti0, blk):
                # transpose O.T block back to natural layout, normalize by
                # the denominator column, store full 512B rows
                out_t = outtp.tile([P, NHC * HD], F32, tag="outt",
                                   name="out_t")
                for h in range(NHC):
                    tr = ps_tr.tile([P, 65], F32, tag="tr", name="tr", bufs=1)
                    nc.tensor.transpose(
                        tr, ot_tiles[h][:, blk * P : (blk + 1) * P],
                        ident[0:65, 0:65],
                    )
                    rc = rcpool.tile([P, 1], F32, tag="rc", name="rc")
                    nc.vector.reciprocal(rc, tr[:, 64:65])
                    nc.vector.tensor_scalar_mul(
                        out_t[:, h * HD : (h + 1) * HD], tr[:, 0:64], rc
                    )
                nc.sync.dma_start(
                    out=out[ti0 + blk * P : ti0 + (blk + 1) * P, :],
                    in_=out_t,
                )

            def heater(n):
                # fp32 matmul = 4 cycles/row: one instruction keeps the PE
                # array busy ~850ns (warm) so the HAM clock gate never sees
                # an "idle" window during the exp-paced attention loop.
                # Own PSUM bank + no readers: depends on nothing recent.
                hp = ps_tr.tile([P, IT], F32, tag="heat", name="heat", bufs=1)
                for _ in range(n):
                    nc.tensor.matmul(
                        hp,
                        lhsT=masks_f[:, 0, 0:P],
                        rhs=masks_f[:, 1, :],
                        start=True,
                        stop=True,
                    )

            # software-pipelined attention: scores for pair p+1 are emitted
            # before exp/AV of pair p, so the PE streams scores while the
            # scalar engine exponentiates the previous pair
            def pair_meta(p_i, njt):
                offs = []
                for u in (0, 1):
                    k = 2 * p_i + u - (njt - 4)
                    offs.append(JT * k if k > 0 else 0)
                return offs, 2 * p_i >= njt - 4

            def emit_scores(p_i, i0, njt):
                offs, diag = pair_meta(p_i, njt)
                sc = [
                    ps_sc.tile([P, 2 * IT], F32, tag="sc", name=f"sc{h}")
                    for h in range(NHC)
                ]
                e = [
                    epool.tile([P, 2 * IT], BF16, tag="e", name=f"e{h}")
                    for h in range(NHC)
                ]
                for u in (0, 1):
                    j = 2 * p_i + u
                    for h in range(NHC):
                        hsl = slice(64 * h, 64 * (h + 1))
                        nc.tensor.matmul(
                            sc[h][:, u * IT + offs[u] : (u + 1) * IT],
                            lhsT=kt[hsl, j * JT : (j + 1) * JT],
                            rhs=qt[hsl, i0 + offs[u] : i0 + IT],
                            start=True,
                            stop=True,
                            tile_position=(64 * h, 0),
                        )
                return sc, e, offs, diag

            def emit_exps(state, p_i, njt):
                sc, e, offs, diag = state
                for h in range(NHC):
                    if not diag:
                        nc.scalar.activation(
                            e[h], sc[h], exp_f, bias=zbias, scale=SCALE
                        )
                    else:
                        for u in (0, 1):
                            k = 2 * p_i + u - (njt - 4)
                            usl = slice(u * IT + offs[u], (u + 1) * IT)
                            nc.scalar.activation(
                                e[h][:, usl], sc[h][:, usl], exp_f,
                                bias=zbias, scale=SCALE,
                            )
                            if k >= 0:
                                nc.vector.tensor_mul(
                                    e[h][:, usl], e[h][:, usl],
                                    masks[:, k, offs[u] : IT],
                                )

            def emit_avs(state, av, p_i, njt):
                _, e, offs, _ = state
                for h in range(NHC):
                    for u in (0, 1):
                        j = 2 * p_i + u
                        nc.tensor.matmul(
                            av[h][:, offs[u] : IT],
                            lhsT=v_sb[:, j, 65 * h : 65 * h + 65],
                            rhs=e[h][:, u * IT + offs[u] : (u + 1) * IT],
                            start=(j == 0),
                            stop=(j == njt - 1),
                        )

            pending = None            # (ot tiles, i0) of the previous range
            project_qk(0)
            for t in range(NI):
                i0 = t * IT
                njt = 4 * (t + 1)         # causal: j-tiles 0..njt-1
                npairs = njt // 2
                av = [
                    ps_av.tile([65, IT], F32, tag="av", name=f"av{h}")
                    for h in range(NHC)
                ]
                state = emit_scores(0, i0, njt)
                for p_i in range(npairs):
                    emit_exps(state, p_i, njt)
                    if pending is not None and p_i < IT // P:
                        tail_block(pending[0], pending[1], p_i)
                        if p_i == IT // P - 1:
                            pending = None
                    heater(1)
                    if p_i == 1 and t + 1 < NI:
                        # prefetch next range's Q/K projection off the
                        # critical path
                        project_qk(t + 1)
                    emit_avs(state, av, p_i, njt)
                    nxt = None
                    if p_i + 1 < npairs:
                        nxt = emit_scores(p_i + 1, i0, njt)
                    state = nxt
                # copy O.T out of PSUM (frees the av accumulators for the
                # next range); defer transpose/normalize into the next
                # range's pair loop
                ot = []
                for h in range(NHC):
                    o = otpool.tile([65, IT], F32, tag="ot", name=f"ot{h}")
                    nc.vector.tensor_copy(o, av[h])
                    ot.append(o)
                pending = (ot, i0)
            for blk in range(IT // P):
                tail_block(pending[0], pending[1], blk)
    return nc


def legalize_waits(nc):
    """This toolchain's walrus allows at most ONE sync-wait per instruction;
    split extra waits onto preceding same-engine NoOps (same trick Tile uses
    for its own wait/update carriers)."""
    nsplit = 0
    for f in nc.m.functions:
        for blk in f.blocks:
            new_insts = []
            for inst in blk.instructions:
                si = getattr(inst, "sync_info", None)
                ow = list(si.on_wait) if (si is not None and si.on_wait) else []
                if len(ow) > 1:
                    for w_i, wcond in enumerate(ow[:-1]):
                        nsplit += 1
                        nop = mybir.InstNoOp(
                            name=f"{inst.name}-wsplit{w_i}",
                            sync_info=mybir.SyncInfo(on_wait=[wcond], on_update=[]),
                            bass_nofuse=True,
                            engine=inst.engine,
                        )
                        new_insts.append(nop)
                    si.on_wait = ow[-1:]
                new_insts.append(inst)
            try:
                blk.instructions[:] = new_insts
            except TypeError:
                blk.instructions = new_insts
    return nsplit


_NC_CACHE = None


def _get_nc():
    global _NC_CACHE
    if _NC_CACHE is None:
        nc = build_nc()
        legalize_waits(nc)
        _NC_CACHE = nc
    return _NC_CACHE


def shard_inputs(inputs, qkv_weights):
    import ml_dtypes

    bf16 = ml_dtypes.bfloat16
    x = np.ascontiguousarray(np.asarray(inputs, dtype=np.float32))
    wf = np.ascontiguousarray(np.asarray(qkv_weights, dtype=np.float32))
    in_maps = []
    for c in range(8):
        b, g = divmod(c, 4)
        lo = g * P
        xt_c = np.ascontiguousarray(x[b].T).astype(bf16)
        w_c = np.ascontiguousarray(
            np.concatenate(
                [wf[:, q * D + lo : q * D + lo + P] for q in range(3)], axis=1
            )
        ).astype(bf16)
        in_maps.append({"xt": xt_c, "w": w_c})
    return in_maps


def gather_outputs(results):
    out = np.empty((B, S, D), dtype=np.float32)
    for c in range(8):
        b, g = divmod(c, 4)
        out[b, :, g * P : (g + 1) * P] = results[c]["out"]
    return out


def run(in_maps, **kwargs):
    from concourse.bass_utils import run_bass_kernel_spmd

    return run_bass_kernel_spmd(_get_nc(), in_maps, list(range(8)), **kwargs)


def kernel(**inputs):
    in_maps = shard_inputs(inputs["inputs"], inputs["qkv_weights"])
    res = run(in_maps)
    return gather_outputs(res.results)


# revision 16
# speedup vs baseline: 1.1539x; 1.1539x over previous
"""Multi-head causal self-attention (B=2, S=4096, D=512, H=8) on 8 trn2 cores.

Sharding: batch*heads = 16 (b,h) pairs -> 2 heads per core (head-parallel,
qkv weight columns sharded per head group). Zero cross-core communication.

Per-core kernel (heads h0=2g, h1=2g+1 stacked on partition halves):
  - inputs: xt = X[b].T  (512, 4096),  w = [Wq|Wk|Wv] head cols (512, 384)
  - QT/KT: (128, 4096) with partitions 0-63 = head0 dims, 64-127 = head1
  - V: natural layout per 128-row j-tile, with an appended ones column so
    the AV matmul also produces the softmax denominator (row 64 of out).
  - scores computed transposed (keys on partitions) so softmax sum comes
    from the ones column; exp on ScalarE with scale=1/8 folded in; causal
    mask via 4 static mask tiles (DVE multiply) + range restriction.
  - AV accumulated in PSUM over j-tiles; result O.T (65, 512) transposed
    back via PE transpose in 128-col blocks; normalized with per-partition
    reciprocal of the denominator column; DMA'd out as full 512B rows.
All matmuls use float32r (full PE rate at free dim >= 256).
"""

import os
import sys

import numpy as np

for _p in ("/opt/trn_rl_repo", "/root/.axon_site/_ro/trn_rl_repo"):
    if os.path.isdir(_p) and _p not in sys.path:
        sys.path.append(_p)

import concourse.bass as bass
import concourse.tile as tile
from concourse import mybir
from concourse.masks import make_identity

F32 = mybir.dt.float32
F32R = mybir.dt.float32r
BF16 = mybir.dt.bfloat16

B, S, D, H = 2, 4096, 512, 8
HD = 64          # head dim
NHC = 2          # heads per core
P = 128          # partitions
KC = D // P      # 4 contraction chunks for the projection
IT = 512         # query-range width
NI = S // IT     # 8 query ranges
JT = 128         # key-tile width
NJ = S // JT     # 32 key tiles
SCALE = 1.0 / np.sqrt(HD)  # 0.125


def build_nc():
    nc = bass.Bass()
    xt = nc.declare_dram_parameter("xt", [D, S], BF16, isOutput=False)
    w = nc.declare_dram_parameter("w", [D, 3 * P], BF16, isOutput=False)
    out = nc.declare_dram_parameter("out", [S, NHC * HD], F32, isOutput=True)

    with tile.TileContext(nc) as tc:
        with (
            tc.tile_pool(name="singles", bufs=1) as singles,
            tc.tile_pool(name="epool", bufs=6) as epool,
            tc.tile_pool(name="otpool", bufs=4) as otpool,
            tc.tile_pool(name="outtp", bufs=3) as outtp,
            tc.tile_pool(name="rcpool", bufs=4) as rcpool,
            tc.tile_pool(name="ps_sc", bufs=2, space="PSUM") as ps_sc,
            tc.tile_pool(name="ps_av", bufs=2, space="PSUM") as ps_av,
            tc.tile_pool(name="ps_tr", bufs=2, space="PSUM") as ps_tr,
        ):
            # ---- resident tensors -------------------------------------
            xt_sb = singles.tile([P, KC, S], BF16, name="xt_sb")
            w_sb = singles.tile([P, KC, 3 * P], BF16, name="w_sb")
            qt = singles.tile([P, S], BF16, name="qt")
            kt = singles.tile([P, S], BF16, name="kt")
            # V per j-tile: [jt, 0:64] head0, [jt, 64] ones, [jt, 65:129]
            # head1, [jt, 129] ones
            v_sb = singles.tile([P, NJ, 130], BF16, name="v_sb")
            masks_f = singles.tile([P, 4, IT], F32, name="masks_f")
            masks = singles.tile([P, 4, IT], BF16, name="masks")
            ident = singles.tile([P, P], F32, name="ident")
            zbias = singles.tile([P, 1], F32, name="zbias")

            # ---- loads + constants ------------------------------------
            nc.sync.dma_start(
                out=w_sb, in_=w[:, :].rearrange("(c p) n -> p c n", p=P)
            )
            xt_r = xt[:, :].rearrange("(c p) s -> p c s", p=P)
            for c in range(KC):
                nc.sync.dma_start(out=xt_sb[:, c, :], in_=xt_r[:, c, :])

            make_identity(nc, ident)
            nc.vector.memset(zbias, 0.0)
            # ones columns for the denominator rows
            nc.vector.memset(v_sb[:, :, 64:65], 1.0)
            nc.vector.memset(v_sb[:, :, 129:130], 1.0)
            # mask k: keep (=1) iff x - p - 128k >= 0, else 0
            for k in range(4):
                nc.gpsimd.memset(masks_f[:, k, :], 1.0)
                nc.gpsimd.affine_select(
                    out=masks_f[:, k, :],
                    in_=masks_f[:, k, :],
                    compare_op=mybir.AluOpType.is_ge,
                    fill=0.0,
                    base=-JT * k,
                    pattern=[[1, IT]],
                    channel_multiplier=-1,
                )
            nc.vector.tensor_copy(masks, masks_f)

            # ---- projections ------------------------------------------
            # V first (small-N matmuls, poor PE duty) so the dense QT/KT
            # burst runs last and flows straight into attention warm.
            for j in range(NJ):
                ps_v = ps_av.tile([P, IT], F32, tag="av", name="ps_v")
                for c in range(KC):
                    nc.tensor.matmul(
                        ps_v[:, 0:P],
                        lhsT=xt_sb[:, c, j * JT : (j + 1) * JT],
                        rhs=w_sb[:, c, 2 * P : 3 * P],
                        start=(c == 0),
                        stop=(c == KC - 1),
                    )
                nc.vector.tensor_copy(v_sb[:, j, 0:64], ps_v[:, 0:64])
                nc.vector.tensor_copy(v_sb[:, j, 65:129], ps_v[:, 64:128])
            # QT/KT: out = w_chunk.T @ xt_chunk  -> [128 head-dims, 512]
            # Emitted just-in-time inside the attention loop (range t needs
            # qt slice t and kt slices <= t) so dense projection matmuls mix
            # into the attention stream instead of a serial phase.
            def project_qk(r):
                sl = slice(r * IT, (r + 1) * IT)
                ps_q = ps_sc.tile([P, 2 * IT], F32, tag="sc", name="ps_q")
                for c in range(KC):
                    nc.tensor.matmul(
                        ps_q[:, 0:IT],
                        lhsT=w_sb[:, c, 0:P],
                        rhs=xt_sb[:, c, sl],
                        start=(c == 0),
                        stop=(c == KC - 1),
                    )
                for c in range(KC):
                    nc.tensor.matmul(
                        ps_q[:, IT : 2 * IT],
                        lhsT=w_sb[:, c, P : 2 * P],
                        rhs=xt_sb[:, c, sl],
                        start=(c == 0),
                        stop=(c == KC - 1),
                    )
                nc.vector.tensor_copy(qt[:, sl], ps_q[:, 0:IT])
                nc.vector.tensor_copy(kt[:, sl], ps_q[:, IT : 2 * IT])

            # ---- attention --------------------------------------------
            exp_f = mybir.ActivationFunctionType.Exp

            def tail_block(ot_tiles, ti0, blk):
                # transpose O.T block back to natural layout, normalize by
                # the denominator column, store full 512B rows
                out_t = outtp.tile([P, NHC * HD], F32, tag="outt",
                                   name="out_t")
                for h in range(NHC):
                    tr = ps_tr.tile([P, 65], F32, tag="tr", name="tr", bufs=1)
                    nc.tensor.transpose(
                        tr, ot_tiles[h][:, blk * P : (blk + 1) * P],
                        ident[0:65, 0:65],
                    )
                    rc = rcpool.tile([P, 1], F32, tag="rc", name="rc")
                    nc.vector.reciprocal(rc, tr[:, 64:65])
                    nc.vector.tensor_scalar_mul(
                        out_t[:, h * HD : (h + 1) * HD], tr[:, 0:64], rc
                    )
                nc.sync.dma_start(
                    out=out[ti0 + blk * P : ti0 + (blk + 1) * P, :],
                    in_=out_t,
                )

            def heater(n):
                # fp32 matmul = 4 cycles/row: one instruction keeps the PE
                # array busy ~850ns (warm) so the HAM clock gate never sees
                # an "idle" window during the exp-paced attention loop.
                # Own PSUM bank + no readers: depends on nothing recent.
                hp = ps_tr.tile([P, IT], F32, tag="heat", name="heat", bufs=1)
                for _ in range(n):
                    nc.tensor.matmul(
                        hp,
                        lhsT=masks_f[:, 0, 0:P],
                        rhs=masks_f[:, 1, :],
                        start=True,
                        stop=True,
                    )

            # software-pipelined attention: scores for pair p+1 are emitted
            # before exp/AV of pair p, so the PE streams scores while the
            # scalar engine exponentiates the previous pair
            def pair_meta(p_i, njt):
                offs = []
                for u in (0, 1):
                    k = 2 * p_i + u - (njt - 4)
                    offs.append(JT * k if k > 0 else 0)
                return offs, 2 * p_i >= njt - 4

            def emit_scores(p_i, i0, njt):
                offs, diag = pair_meta(p_i, njt)
                sc = [
                    ps_sc.tile([P, 2 * IT], F32, tag="sc", name=f"sc{h}")
                    for h in range(NHC)
                ]
                e = [
                    epool.tile([P, 2 * IT], BF16, tag="e", name=f"e{h}")
                    for h in range(NHC)
                ]
                for u in (0, 1):
                    j = 2 * p_i + u
                    for h in range(NHC):
                        hsl = slice(64 * h, 64 * (h + 1))
                        nc.tensor.matmul(
                            sc[h][:, u * IT + offs[u] : (u + 1) * IT],
                            lhsT=kt[hsl, j * JT : (j + 1) * JT],
                            rhs=qt[hsl, i0 + offs[u] : i0 + IT],
                            start=True,
                            stop=True,
                            tile_position=(64 * h, 0),
                        )
                return sc, e, offs, diag

            def emit_exps(state, p_i, njt):
                sc, e, offs, diag = state
                for h in range(NHC):
                    if not diag:
                        nc.scalar.activation(
                            e[h], sc[h], exp_f, bias=zbias, scale=SCALE
                        )
                    else:
                        for u in (0, 1):
                            k = 2 * p_i + u - (njt - 4)
                            usl = slice(u * IT + offs[u], (u + 1) * IT)
                            nc.scalar.activation(
                                e[h][:, usl], sc[h][:, usl], exp_f,
                                bias=zbias, scale=SCALE,
                            )
                            if k >= 0:
                                nc.vector.tensor_mul(
                                    e[h][:, usl], e[h][:, usl],
                                    masks[:, k, offs[u] : IT],
                                )

            def emit_avs(state, av, p_i, njt):
                _, e, offs, _ = state
                for h in range(NHC):
                    for u in (0, 1):
                        j = 2 * p_i + u
                        nc.tensor.matmul(
                            av[h][:, offs[u] : IT],
                            lhsT=v_sb[:, j, 65 * h : 65 * h + 65],
                            rhs=e[h][:, u * IT + offs[u] : (u + 1) * IT],
                            start=(j == 0),
                            stop=(j == njt - 1),
                        )

            pending = None            # (ot tiles, i0) of the previous range
            project_qk(0)
            for t in range(NI):
                i0 = t * IT
                njt = 4 * (t + 1)         # causal: j-tiles 0..njt-1
                npairs = njt // 2
                av = [
                    ps_av.tile([65, IT], F32, tag="av", name=f"av{h}")
                    for h in range(NHC)
                ]
                state = emit_scores(0, i0, njt)
                for p_i in range(npairs):
                    emit_exps(state, p_i, njt)
                    if pending is not None and p_i < IT // P:
                        tail_block(pending[0], pending[1], p_i)
                        if p_i == IT // P - 1:
                            pending = None
                    if p_i == 1 and t + 1 < NI:
                        # prefetch next range's Q/K projection off the
                        # critical path
                        project_qk(t + 1)
                    emit_avs(state, av, p_i, njt)
                    nxt = None
                    if p_i + 1 < npairs:
                        nxt = emit_scores(p_i + 1, i0, njt)
                    state = nxt
                # copy O.T out of PSUM (frees the av accumulators for the
                # next range); defer transpose/normalize into the next
                # range's pair loop
                ot = []
                for h in range(NHC):
                    o = otpool.tile([65, IT], F32, tag="ot", name=f"ot{h}")
                    nc.vector.tensor_copy(o, av[h])
                    ot.append(o)
                pending = (ot, i0)
            for blk in range(IT // P):
                tail_block(pending[0], pending[1], blk)
    return nc


def legalize_waits(nc):
    """This toolchain's walrus allows at most ONE sync-wait per instruction;
    split extra waits onto preceding same-engine NoOps (same trick Tile uses
    for its own wait/update carriers)."""
    nsplit = 0
    for f in nc.m.functions:
        for blk in f.blocks:
            new_insts = []
            for inst in blk.instructions:
                si = getattr(inst, "sync_info", None)
                ow = list(si.on_wait) if (si is not None and si.on_wait) else []
                if len(ow) > 1:
                    for w_i, wcond in enumerate(ow[:-1]):
                        nsplit += 1
                        nop = mybir.InstNoOp(
                            name=f"{inst.name}-wsplit{w_i}",
                            sync_info=mybir.SyncInfo(on_wait=[wcond], on_update=[]),
                            bass_nofuse=True,
                            engine=inst.engine,
                        )
                        new_insts.append(nop)
                    si.on_wait = ow[-1:]
                new_insts.append(inst)
            try:
                blk.instructions[:] = new_insts
            except TypeError:
                blk.instructions = new_insts
    return nsplit


_NC_CACHE = None


def _get_nc():
    global _NC_CACHE
    if _NC_CACHE is None:
        nc = build_nc()
        legalize_waits(nc)
        _NC_CACHE = nc
    return _NC_CACHE


def shard_inputs(inputs, qkv_weights):
    import ml_dtypes

    bf16 = ml_dtypes.bfloat16
    x = np.ascontiguousarray(np.asarray(inputs, dtype=np.float32))
    wf = np.ascontiguousarray(np.asarray(qkv_weights, dtype=np.float32))
    in_maps = []
    for c in range(8):
        b, g = divmod(c, 4)
        lo = g * P
        xt_c = np.ascontiguousarray(x[b].T).astype(bf16)
        w_c = np.ascontiguousarray(
            np.concatenate(
                [wf[:, q * D + lo : q * D + lo + P] for q in range(3)], axis=1
            )
        ).astype(bf16)
        in_maps.append({"xt": xt_c, "w": w_c})
    return in_maps


def gather_outputs(results):
    out = np.empty((B, S, D), dtype=np.float32)
    for c in range(8):
        b, g = divmod(c, 4)
        out[b, :, g * P : (g + 1) * P] = results[c]["out"]
    return out


def run(in_maps, **kwargs):
    from concourse.bass_utils import run_bass_kernel_spmd

    return run_bass_kernel_spmd(_get_nc(), in_maps, list(range(8)), **kwargs)


def kernel(**inputs):
    in_maps = shard_inputs(inputs["inputs"], inputs["qkv_weights"])
    res = run(in_maps)
    return gather_outputs(res.results)
